# revision 1
# baseline (speedup 1.0000x reference)
"""ConvTranspose3d (C_in=128, C_out=64, k=4, stride=2, pad=1) on 8 Trainium2
NeuronCores, optimized for end-to-end latency over the axon tunnel.

The axon client<->terminal link moves ~40MB/s with ~75ms per RPC, so the
kernel is designed around minimizing transferred bytes and round trips:

- Sharding: core <-> (batch n, output-H quarter ht). Each core computes the
  full depth/width for oh in [16ht, 16ht+16), i.e. ALL (od%2, oh%2, ow%2)
  parity classes, so every core runs a truly identical program (no per-core
  frame shifts) and inputs need no host-side specialization; h-quarters
  have the cheapest halo (10/8 rows duplicated).
- x crosses the tunnel as int8 (5.25MB): host-quantized with one global
  scale and exact rounding, converted to bf16 on-chip (exact for
  -127..127). The scale folds into the host-side output dequant (the
  in-kernel output quantization is scale-invariant) and the bias is added
  on the host, so nothing per-call ships besides x itself.
- Weight stacks are tiny, replicated, and cached on-device keyed by
  content hash: warm calls skip their transfer entirely.
- The bass kernel quantizes its own output: fp32 PSUM, ACT drains write a
  parity-interleaved [co|od|oh|ow] SBUF layout, |y| -> per-partition max ->
  reciprocal -> one vector pass emits int8; 16.8MB of int8 + 128
  scales/core come back. End-to-end rel err ~1.4e-2 vs the 2e-2 gate.
- All jax callables are built once and cached; the previous call's fetched
  outputs are donated back as the custom call's output buffers, so no
  zero-buffers ever cross the tunnel. Fetches are per-shard and overlap
  the host dequant/scatter; the tiny amax fetches ride ahead of them.

Polyphase math: od = 2*id - 1 + kd. For od = 2m+rd: rd=0 takes kd in {1,3}
(id = m, m-1), rd=1 takes kd in {0,2} (id = m+1, m); same along h and w.
PSUM partitions pack (rd*64 + co) so each matmul contracts the full Cin=128
and produces 128 outputs. Per (rh,rw) combo and output pair m: 12 taps =
3 d-groups x 2 h-options x 2 w-options (d-groups +-1 use a half-zero lhsT).

Cross-engine scheduling follows the proven baseline idioms: dummy bf16
ldweights make the PE observe input DMAs (so matmuls carry only their PSUM
WAR wait), drains funnel through ACT, and the Tile tail drain is split per
engine (walrus rejects instructions with >1 sem wait).
"""
import hashlib
from concurrent.futures import ThreadPoolExecutor

import numpy as np
import ml_dtypes

import jax
import jax.numpy as jnp
from jax.sharding import Mesh, PartitionSpec, NamedSharding

import concourse.bass as bass
import concourse.mybir as mybir
import concourse.tile as tile
from concourse.bass2jax import (
    _bass_exec_p,
    install_neuronx_cc_hook,
    partition_id_tensor,
)

F32 = mybir.dt.float32
BF16 = mybir.dt.bfloat16
ACT_COPY = mybir.ActivationFunctionType.Copy
IDENT = mybir.ActivationFunctionType.Identity
BF16_NP = ml_dtypes.bfloat16

N_BATCH, C_IN, C_OUT = 2, 128, 64
N_D, N_HW = 16, 32  # input spatial grid
N_CORES = 8
SLAB = 4  # input-depth planes owned per core (output od slab = 8)

# kernel index along one dim for output parity r and shift option i:
# ih = j + delta, kd = r + 1 - 2*delta;  r=0: deltas (0,-1) -> k (1,3);
# r=1: deltas (+1,0) -> k (0,2). Row base in the 1-padded frame = 1 + delta.
_DELTAS = {0: (0, -1), 1: (1, 0)}


class _SplitDrainTileContext(tile.TileContext):
    """TileContext whose kernel-tail drain is split into one drain per proc
    (this walrus build rejects instructions carrying more than ~2 sync
    commands, and the stock tail drain waits on every active proc)."""

    def _drain_and_barrier(self, tick_clock, wait_clock):
        from concourse.vector_clock import ScopedClock, VectorClock

        gc = tick_clock.global_clock
        n = len(gc)
        for i in range(n):
            t = gc[i]
            if t <= 0:
                continue
            vc = VectorClock([0] * n)
            vc.require_at_least(i, t)
            d = self.nc.sync.drain()
            wait_clock.add_sem_waits(d.ins, ScopedClock({None: vc}))
        self.nc.all_engine_barrier()
        assert self.sems is not None
        popped = self.nc._tile_sem_poison_stack.pop()
        assert popped is self._sem_poison
        self.nc.clear_and_free_semaphores(list(self.sems.allocated().values()))
        self.nc.all_engine_barrier()


def _build_program():
    nc = bass.Bass()
    # 16 depth planes x 10 h-rows (rows [8*ht-1, 8*ht+9) of the h-quarter,
    # OOB rows zero); h-quarter sharding halves the halo overhead vs
    # depth-slab sharding (10/8 vs 6/4 duplication)
    # x ships int8 (host-quantized, global scale folded into the host-side
    # dequant); the on-chip int8->bf16 convert is exact for -127..127
    xin = nc.declare_dram_parameter("xin", [C_IN, 16, 10, 32], mybir.dt.int8, isOutput=False)
    wt_in = nc.declare_dram_parameter("wt", [C_IN, 48 * 128], BF16, isOutput=False)
    b_in = nc.declare_dram_parameter("b2", [128, 1], F32, isOutput=False)
    # yq[co, m, rd, (oh_loc ow)]: od = 2*m + rd, so the host view
    # [64, 32, 16, 64] is the plain [co, od, oh_quarter, ow] slab
    yq_out = nc.declare_dram_parameter("yq", [C_OUT, 16, 2, 1024], mybir.dt.int8, isOutput=True)
    # per-partition (rd*64+co) abs-max of y: the int8 scale is amax/127
    am_out = nc.declare_dram_parameter("amax", [128, 1], F32, isOutput=True)

    with _SplitDrainTileContext(nc) as tc:
        with (
            tc.tile_pool(name="const", bufs=1) as cpool,
            tc.tile_pool(name="xs", bufs=1) as xpool,
            tc.tile_pool(name="ps", bufs=4, space="PSUM") as pspool,
        ):
            lw = cpool.tile([128, 48 * 128], BF16)
            nc.sync.dma_start(lw[:], wt_in[:])
            # PE observes the weight DMA once, so matmuls never carry a DMA
            # wait on top of their PSUM-WAR wait (1-wait walrus budget)
            nc.tensor.ldweights(lw[:, 0:1])

            br = cpool.tile([128, 1], F32)
            nc.sync.dma_start(br[:], b_in[:])
            bia = cpool.tile([128, 1], F32)
            nc.scalar.activation(bia[:], br[:], ACT_COPY)
            # absorb the ACT-pipeline self-wait on bia once, so drains below
            # only ever wait on PE
            obs = cpool.tile([128, 1], F32)
            nc.scalar.activation(obs[:], bia[:], ACT_COPY)

            # int8 staging tile (unpadded, one flat DMA), then a single
            # vector convert writes bf16 into the zero-padded frame interior
            xq = xpool.tile([128, 16, 10, 32], mybir.dt.int8, name="xq", tag="xq")
            xt = xpool.tile([128, 18, 10, 34], BF16, name="xt", tag="xt")
            # zero only the pad borders: overlapping the converts' output
            # ranges would add same-engine pipeline waits to them
            nc.vector.memset(xt[:, 0], 0)
            nc.vector.memset(xt[:, 17], 0)
            nc.vector.memset(xt[:, 1:17, :, 0:1], 0)
            nc.vector.memset(xt[:, 1:17, :, 33:34], 0)
            for g in range(4):
                # 4-plane groups: each DMA lands on one queue, so each
                # convert carries exactly its own DMA wait (1-wait budget)
                nc.sync.dma_start(
                    xq[:, 4 * g : 4 * g + 4].rearrange("p d r c -> p (d r c)"),
                    xin[:, 4 * g : 4 * g + 4].rearrange("p d r c -> p (d r c)"),
                )
                nc.vector.tensor_scalar(
                    xt[:, 1 + 4 * g : 5 + 4 * g, :, 1:33],
                    xq[:, 4 * g : 4 * g + 4],
                    1.0,
                    None,
                    mybir.AluOpType.mult,
                )
            # PE observes the vector engine once after the last convert
            # (covers the memset and all converts); matmuls then carry only
            # their PSUM-WAR wait (1-wait walrus budget)
            nc.tensor.ldweights(xt[:, 1, 0, 1:2])

            # y in fp32, laid out so (rh, rw) interleave happens at drain
            # time: dims (m, j', rh, l, rw) <-> [m, oh_loc, ow]
            out_sb = cpool.tile([128, 16, 8, 2, 32, 2], F32)

            for q in range(4):  # combo (rh, rw)
                rh, rw = q // 2, q % 2
                for m in range(16):  # output pair: od = 2m + rd
                    pst = pspool.tile(
                        [128, 8, 32], F32, name=f"ps_{q}_{m}", tag="ps"
                    )
                    for t in range(12):
                        dgi, hi, wi = t // 4, (t // 2) % 2, t % 2
                        # d-group: 0 -> id=m (both rd), 1 -> id=m-1
                        # (rd=0 cols), 2 -> id=m+1 (rd=1 cols)
                        p = m + (1, 0, 2)[dgi]
                        rb = 1 + _DELTAS[rh][hi]
                        cb = 1 + _DELTAS[rw][wi]
                        nc.tensor.matmul(
                            pst[:],
                            lw[:, (q * 12 + t) * 128 : (q * 12 + t + 1) * 128],
                            xt[:, p, rb : rb + 8, cb : cb + 32],
                            start=(t == 0),
                            stop=(t == 11),
                        )
                    # no bias here: y' = W @ xq is in quantized-x units;
                    # the host adds bias after rescaling by s
                    nc.scalar.activation(
                        out_sb[:, m, :, rh, :, rw],
                        pst[:],
                        IDENT,
                    )

            # per-partition int8 quantization: |y| on ACT (same engine as the
            # drains, so no extra sync), top-8 max on vector, reciprocal,
            # then one vector pass writes y * (1/amax) * 127 as int8. Host
            # recovers the scale as amax/127.
            yf = out_sb[:].rearrange("p m j h l w -> p (m j h l w)")  # 16384
            abs_t = cpool.tile([128, 16384], F32)
            nc.scalar.activation(
                abs_t[:], yf, mybir.ActivationFunctionType.Abs
            )
            mx8 = cpool.tile([128, 8], F32)
            nc.vector.max(mx8[:], abs_t[:])
            rs = cpool.tile([128, 1], F32)
            nc.vector.reciprocal(rs[:], mx8[:, 0:1])
            q8 = cpool.tile([128, 16, 1024], mybir.dt.int8)
            nc.vector.tensor_scalar(
                q8[:].rearrange("p m f -> p (m f)"),
                yf,
                rs[:],
                127.0,
                mybir.AluOpType.mult,
                mybir.AluOpType.mult,
            )
            # outputs: rd split is the partition halves; both DMAs write
            # fully contiguous 4KB bursts per (co, lm)
            nc.gpsimd.dma_start(am_out[:], mx8[:, 0:1])
            for rd in range(2):
                nc.gpsimd.dma_start(
                    yq_out[:, :, rd, :], q8[64 * rd : 64 * rd + 64]
                )
    # extended-inst bass methods (tensor_tensor_reduce) leave .instr empty;
    # codegen them now or walrus fails with "ISA wrong length"
    from concourse.library_overlay import lower_extended_insts

    lower_extended_insts(nc)
    return nc


# ---------------------------------------------------------------------------
# host <-> device runtime (built once, cached)
# ---------------------------------------------------------------------------

_RT: dict | None = None


def _get_runtime():
    global _RT
    if _RT is not None:
        return _RT
    install_neuronx_cc_hook()
    nc = _build_program()

    partition_name = (
        nc.partition_id_tensor.name if nc.partition_id_tensor is not None else None
    )
    in_names, out_names, out_avals = [], [], []
    for alloc in nc.m.functions[0].allocations:
        if not isinstance(alloc, mybir.MemoryLocationSet):
            continue
        name = alloc.memorylocations[0].name
        if alloc.kind == "ExternalInput":
            if name != partition_name:
                in_names.append(name)
        elif alloc.kind == "ExternalOutput":
            out_names.append(name)
            out_avals.append(
                jax.core.ShapedArray(
                    tuple(alloc.tensor_shape), mybir.dt.np(alloc.dtype)
                )
            )
    assert in_names == ["xin", "wt", "b2"], in_names
    assert out_names == ["yq", "amax"], out_names

    devices = jax.devices()[:N_CORES]
    mesh = Mesh(np.asarray(devices), ("core",))
    shard = NamedSharding(mesh, PartitionSpec("core"))

    all_in_names = tuple(in_names) + tuple(out_names)
    if partition_name is not None:
        all_in_names = all_in_names + (partition_name,)

    def _body(xin, wt, b2, yq_buf, am_buf):
        operands = [xin, wt, b2, yq_buf, am_buf]
        if partition_name is not None:
            operands.append(partition_id_tensor())
        outs = _bass_exec_p.bind(
            *operands,
            out_avals=tuple(out_avals),
            in_names=all_in_names,
            out_names=tuple(out_names),
            lowering_input_output_aliases=(),
            sim_require_finite=True,
            sim_require_nnan=True,
            nc=nc,
        )
        return tuple(outs)

    P = PartitionSpec
    bass_jit = jax.jit(
        jax.shard_map(
            _body,
            mesh=mesh,
            in_specs=(P("core"), P(), P(), P("core"), P("core")),
            out_specs=(P("core"), P("core")),
            check_vma=False,
        ),
        donate_argnums=(3, 4),
        keep_unused=True,
    )

    zeros_jit = jax.jit(
        lambda: (
            jnp.zeros((N_CORES * C_OUT, 16, 2, 1024), jnp.int8),
            jnp.zeros((N_CORES * 128, 1), jnp.float32),
        ),
        out_shardings=(shard, shard),
    )

    _RT = {
        "nc": nc,
        "mesh": mesh,
        "shard": shard,
        "repl": NamedSharding(mesh, PartitionSpec()),
        "bass_jit": bass_jit,
        "zeros_jit": zeros_jit,
        "wcache_key": None,
        "wcache_val": None,
        "obuf": None,
        "xbuf_host": np.zeros((N_CORES, C_IN, 16, 10, 32), np.int8),
        "pool": ThreadPoolExecutor(3),
    }
    return _RT


def _build_w_stack(weight):
    """[128, 48*128] bf16: 4 combos x 12 taps of lhsT, cols (rd*64 + co)."""
    stack = np.zeros((4, 12, C_IN, 128), np.float32)
    for q in range(4):
        rh, rw = q // 2, q % 2
        for t in range(12):
            dgi, hi, wi = t // 4, (t // 2) % 2, t % 2
            kh = rh + 1 - 2 * _DELTAS[rh][hi]
            kw = rw + 1 - 2 * _DELTAS[rw][wi]
            L = stack[q, t]
            if dgi == 0:
                L[:, 0:64] = weight[:, :, 1, kh, kw]
                L[:, 64:128] = weight[:, :, 2, kh, kw]
            elif dgi == 1:
                L[:, 0:64] = weight[:, :, 3, kh, kw]
            else:
                L[:, 64:128] = weight[:, :, 0, kh, kw]
    return (
        np.transpose(stack, (2, 0, 1, 3)).reshape(C_IN, 48 * 128).astype(BF16_NP)
    )


def _device_weights(rt, weight, bias):
    key = hashlib.blake2b(
        weight.tobytes() + bias.tobytes(), digest_size=16
    ).digest()
    if rt["wcache_key"] == key:
        return rt["wcache_val"]
    wt = _build_w_stack(weight)
    b2 = np.concatenate([bias, bias]).astype(np.float32).reshape(128, 1)
    w_dev = jax.device_put(wt, rt["repl"])
    b_dev = jax.device_put(b2, rt["repl"])
    rt["wcache_key"] = key
    rt["wcache_val"] = (w_dev, b_dev)
    return w_dev, b_dev


def kernel(x, weight, bias):
    x = np.asarray(x, dtype=np.float32)
    weight = np.asarray(weight, dtype=np.float32)
    bias = np.asarray(bias, dtype=np.float32)
    rt = _get_runtime()

    w_dev, b_dev = _device_weights(rt, weight, bias)

    # host-quantize x to int8 with one global scale (exact rounding here;
    # the device convert back to bf16 is exact), then slice h-quarter rows
    # [8*ht-1, 8*ht+9) per core (1-row halo each side, OOB stays zero)
    xs = np.abs(x).max() / np.float32(127.0)
    xb = np.clip(np.rint(x * (np.float32(1.0) / xs)), -127, 127).astype(np.int8)
    buf = rt["xbuf_host"]
    for c in range(N_CORES):
        n, ht = c // 4, c % 4
        lo = 8 * ht - 1
        glo, ghi = max(lo, 0), min(lo + 10, N_HW)
        buf[c, :, :, glo - lo : glo - lo + (ghi - glo)] = xb[n, :, :, glo:ghi]
    x_dev = jax.device_put(buf.reshape(N_CORES * C_IN, 16, 10, 32), rt["shard"])

    if rt["obuf"] is None:
        rt["obuf"] = rt["zeros_jit"]()
    yq_dev, am_dev = rt["bass_jit"](x_dev, w_dev, b_dev, *rt["obuf"])
    # fetched arrays stay valid device buffers: recycle them as next call's
    # donated output buffers (their contents are fully overwritten)
    rt["obuf"] = (yq_dev, am_dev)

    # fetch per-core shards and overlap the host dequant/scatter with the
    # remaining transfers; the tiny amax fetch rides ahead of the first shard
    pool = rt["pool"]
    am_fut = pool.submit(np.asarray, am_dev)
    shard_futs = {}
    for sh in yq_dev.addressable_shards:
        c = sh.index[0].start // C_OUT
        shard_futs[c] = pool.submit(np.asarray, sh.data)
    am = am_fut.result().reshape(N_CORES, 2, C_OUT)

    y = np.empty((N_BATCH, C_OUT, 2 * N_D, 2 * N_HW, 2 * N_HW), np.float32)
    for c in range(N_CORES):
        n, ht = c // 4, c % 4
        yqc = shard_futs[c].result().reshape(C_OUT, 16, 2, 16, 64)
        # scale for (co, m, rd, .) = xscale * amax[c, rd, co] / 127, and the
        # bias (not added on-chip) comes in here
        sc = (am[c].transpose(1, 0) * (xs / np.float32(127.0))).astype(np.float32)
        deq = yqc * sc[:, None, :, None, None] + bias[:, None, None, None, None]
        y[n, :, :, 16 * ht : 16 * ht + 16, :] = deq.reshape(C_OUT, 32, 16, 64)
    return y

